# revision 2
# baseline (speedup 1.0000x reference)
"""Trainium2 Bass kernel for DiscreteLinear (MoE-style routed linear).

Computes z[b, :] = x[b, :] @ weight[a[b]].T + bias[a[b]] for
B=2048 tokens, D=512 features, A=64 expert matrices.

Strategy (expert parallelism, 8 NeuronCores):
- Host routes tokens by expert (stable argsort of `a`), assigns 8 experts
  per core, and packs each core's inputs:
    * x gathered per expert, transposed to [D, tokens] and laid out as
      [128, JB * VE * C] SBUF-ready tiles (JB=4 contraction blocks of 128,
      VE virtual experts per core, C token capacity per virtual expert),
    * weight transposed per expert to [j, i] and laid out [VE, 128, JB*D],
    * bias [1, VE*D].
- Device, per virtual expert: PSUM-accumulated matmuls over the 4
  contraction blocks (out[t, i] = sum_j xT[j, t] * wT[j, i]) plus a k=1
  matmul (ones[1, C] x bias[1, D]) that fuses the bias add into the same
  accumulation; DVE copies PSUM->SBUF; DMA writes [C, D] rows out.
- Host scatters rows back through the inverse permutation.

Each expert's 1 MB weight matrix is read from HBM exactly once across the
whole machine (the minimum possible), ~8 MB per core.
"""

import numpy as np

import concourse.bass as bass  # noqa: F401  (registers engines on import)
import concourse.tile as tile
from concourse import bacc, mybir
from concourse.bass_utils import run_bass_kernel_spmd

N_CORES = 8
P = 128                    # SBUF partitions / matmul contraction tile
A, D, B = 64, 512, 2048    # experts, feature dim, batch
JB = D // P                # contraction blocks per matmul group (4)
EPC = A // N_CORES         # experts owned per core (8)

# Matmul operand dtype. float32r is fp32 data with the PE's reduced-precision
# full-rate mode; float32 is exact but 4x slower; bfloat16 halves DMA bytes.
MM_DT = mybir.dt.float32
_NP_DT = {mybir.dt.float32: np.float32, mybir.dt.float32r: np.float32}

_build_cache = {}


def _np_dtype(mm_dt):
    if mm_dt in _NP_DT:
        return _NP_DT[mm_dt]
    import ml_dtypes

    return ml_dtypes.bfloat16


def _build(ve, cap, mm_dt):
    """Compile the SPMD program for `ve` virtual experts of capacity `cap`."""
    key = (ve, cap, mm_dt)
    if key in _build_cache:
        return _build_cache[key]

    nc = bacc.Bacc(
        "TRN2", target_bir_lowering=False, debug=False, num_devices=N_CORES
    )
    xt_d = nc.dram_tensor(
        "xt", [P, JB * ve * cap], mm_dt, kind="ExternalInput"
    ).ap()
    w_d = nc.dram_tensor("w", [ve, P, JB * D], mm_dt, kind="ExternalInput").ap()
    b_d = nc.dram_tensor("b", [1, ve * D], mm_dt, kind="ExternalInput").ap()
    z_d = nc.dram_tensor(
        "z", [ve * cap, D], mybir.dt.float32, kind="ExternalOutput"
    ).ap()

    with tile.TileContext(nc) as tc:
        with (
            tc.tile_pool(name="xtp", bufs=1) as xt_pool,
            tc.tile_pool(name="wp", bufs=ve) as w_pool,
            tc.tile_pool(name="bp", bufs=1) as b_pool,
            tc.tile_pool(name="onep", bufs=1) as one_pool,
            tc.tile_pool(name="zp", bufs=4) as z_pool,
            tc.tile_pool(name="psp", bufs=8, space="PSUM") as ps_pool,
        ):
            xt = xt_pool.tile([P, JB * ve * cap], mm_dt)
            nc.sync.dma_start(out=xt[:], in_=xt_d[:])
            bias_t = b_pool.tile([1, ve * D], mm_dt)
            nc.sync.dma_start(out=bias_t[:], in_=b_d[:])
            ones = one_pool.tile([1, cap], mm_dt)
            nc.any.memset(ones[:], 1.0)

            wts = []
            for v in range(ve):
                wt = w_pool.tile([P, JB * D], mm_dt)
                nc.sync.dma_start(out=wt[:], in_=w_d[v])
                wts.append(wt)

            for v in range(ve):
                ps = ps_pool.tile([cap, D], mybir.dt.float32)
                for jb in range(JB):
                    nc.tensor.matmul(
                        ps[:],
                        xt[:, (jb * ve + v) * cap : (jb * ve + v + 1) * cap],
                        wts[v][:, jb * D : (jb + 1) * D],
                        start=(jb == 0),
                        stop=False,
                    )
                nc.tensor.matmul(
                    ps[:],
                    ones[:1, :cap],
                    bias_t[:1, v * D : (v + 1) * D],
                    start=False,
                    stop=True,
                )
                zt = z_pool.tile([cap, D], mybir.dt.float32)
                nc.vector.tensor_copy(zt[:], ps[:])
                nc.sync.dma_start(out=z_d[v * cap : (v + 1) * cap, :], in_=zt[:])

    nc.compile()
    _build_cache[key] = nc
    return nc


def _route(a):
    """Group token indices by expert, split groups into chunks of <=128.

    Returns per-core lists of (expert, token_index_array) virtual experts,
    padded so every core has the same count, plus the shared capacity.
    """
    perm = np.argsort(a, kind="stable")
    counts = np.bincount(a, minlength=A)
    offs = np.concatenate([[0], np.cumsum(counts)])

    vlists = []
    for c in range(N_CORES):
        vl = []
        for e in range(c * EPC, (c + 1) * EPC):
            idx = perm[offs[e] : offs[e + 1]]
            for s in range(0, len(idx), P):
                vl.append((e, idx[s : s + P]))
        vlists.append(vl)

    ve = max(len(vl) for vl in vlists)
    ve = max(ve, 1)
    empty = np.empty(0, dtype=perm.dtype)
    for c, vl in enumerate(vlists):
        while len(vl) < ve:
            vl.append((c * EPC, empty))
    cap = max(max((len(idx) for _, idx in vl), default=1) for vl in vlists)
    cap = max(cap, 1)
    return vlists, ve, cap


def kernel(x, a, weight, bias):
    x = np.ascontiguousarray(np.asarray(x, dtype=np.float32))
    a = np.asarray(a).astype(np.int64)
    weight = np.asarray(weight, dtype=np.float32)
    bias = np.asarray(bias, dtype=np.float32)
    b_, d_ = x.shape
    assert (b_, d_) == (B, D) and weight.shape == (A, D, D)

    np_dt = _np_dtype(MM_DT)
    vlists, ve, cap = _route(a)
    nc = _build(ve, cap, MM_DT)

    # wT[e][j, i] = weight[e][i, j], tiled to [A, 128, JB*D] with the
    # contraction index j = jb*128 + p mapped to (free-block jb, partition p).
    w_pack = np.ascontiguousarray(
        weight.transpose(0, 2, 1)
        .reshape(A, JB, P, D)
        .transpose(0, 2, 1, 3)
        .reshape(A, P, JB * D)
        .astype(np_dt)
    )

    in_maps = []
    for c in range(N_CORES):
        vl = vlists[c]
        xt_arr = np.zeros((P, JB, ve, cap), dtype=np_dt)
        w_arr = np.empty((ve, P, JB * D), dtype=np_dt)
        b_arr = np.empty((1, ve * D), dtype=np_dt)
        for v, (e, idx) in enumerate(vl):
            n = len(idx)
            if n:
                # x[idx].T -> [D, n]; split j into (jb, p)
                xt_arr[:, :, v, :n] = (
                    x[idx].T.reshape(JB, P, n).transpose(1, 0, 2).astype(np_dt)
                )
            w_arr[v] = w_pack[e]
            b_arr[0, v * D : (v + 1) * D] = bias[e].astype(np_dt)
        in_maps.append(
            {
                "xt": np.ascontiguousarray(xt_arr.reshape(P, JB * ve * cap)),
                "w": w_arr,
                "b": b_arr,
            }
        )

    global _last_in_maps
    _last_in_maps = in_maps
    res = run_bass_kernel_spmd(nc, in_maps, list(range(N_CORES)))

    z = np.empty((B, D), dtype=np.float32)
    for c in range(N_CORES):
        z_pad = res.results[c]["z"]
        for v, (e, idx) in enumerate(vlists[c]):
            n = len(idx)
            if n:
                z[idx] = z_pad[v * cap : v * cap + n]
    return z
